# revision 10
# baseline (speedup 1.0000x reference)
"""AdaLN transformer block on 8 TRN2 NeuronCores (Bass/Tile).

Sharding: 4096 tokens (B*S) split 8 ways -> 512 tokens/core; cores (2b, 2b+1)
own batch element b. All per-token compute (LN, adaLN modulation, projections,
FFN) is perfectly sharded, zero duplication. Attention needs full-sequence K/V
per batch element: each core computes K^T / V(+ones col) for its own tokens in
bf16 and a pairwise AllGather (replica groups [[0,1],[2,3],[4,5],[6,7]])
replicates them within each batch pair. The gather output layout is
core-independent: slot 0 = the batch's first 512 tokens, slot 1 = second half.

Precision: all matmuls bf16 (weights pre-cast to bf16 on the host), fp32
accumulation in PSUM. The f32 spine (inputs, LN stats, residuals, softmax
denominators) keeps the error small.

Attention is software-pipelined over a flat 64-iteration space (8 head-pairs x
8 key-tiles): scores(it) -> exp(it-1) -> PV(it-2), so the in-order PE queue
never stalls on the scores->exp->PV dependency chain. Each iteration's two
score matmuls land in one 2-bank PSUM tile [128,1024]; softmax exp is one op
per iteration, alternating between ScalarE (AF.Exp) and VectorE (Schraudolph
bit-trick: int16(x*184.665+16249) reinterpreted as bf16). Query-row masking
(+the 1/sqrt(dh) scale) is folded into the q projection eviction. Softmax
denominators come from an appended ones-column in V (M=65 PV matmuls).
"""
import os
import numpy as np

import concourse.bass as bass
import concourse.bacc as bacc
import concourse.tile as tile
import concourse.mybir as mybir
from concourse import bass_utils

B, S, H, NH = 4, 1024, 1024, 16
DH = H // NH               # 64
EPS = 1e-5
NC = 8
T = (B * S) // NC          # 512 tokens per core
TC = T // 128              # 4
HC = H // 128              # 8
VA_W = NH * (DH + 1)       # 1040
KT_LEN = H * T             # 524288 elements
KT_H = (H // 2) * T        # 262144: half the heads' k^T
VA_H = T * (VA_W // 2)     # 266240: half the heads' v_aug
HALF_LEN = KT_H + VA_H
VA_LEN = T * VA_W          # 532480 elements
RANK_LEN = KT_LEN + VA_LEN

F32 = mybir.dt.float32
BF16 = mybir.dt.bfloat16
I16 = mybir.dt.int16
I32 = mybir.dt.int32
AF = mybir.ActivationFunctionType
OP = mybir.AluOpType

# Schraudolph fast-exp constants for bf16 bit-pattern output (int16 trunc).
EXP_A = 184.6650390625     # 2^7 / ln(2)
EXP_B = 16249.0            # 127*2^7 - c (c tuned, incl. +0.5 trunc comp)
DVE_EXP = True             # half of softmax exps on VectorE via bit-trick

W_NAMES = ["Wsq", "Wsk", "Wsv", "Wso", "Wcq", "Wck", "Wcv", "Wco",
           "ffn_w1", "ffn_w2"]
B_NAMES = ["bsq", "bsk", "bsv", "bso", "bcq", "bck", "bcv", "bco",
           "ffn_b1", "ffn_b2", "ada_b"]
LN_NAMES = ["ln2", "ln3", "lnh", "lnf"]

LAST_RESULT = None
_BUILD_CACHE = {}


def _build(flags):
    use_bias, use_affine = flags
    nc = bacc.Bacc("TRN2", target_bir_lowering=False, debug=False,
                   num_devices=NC)
    ext = {}
    for nm in ["x", "h", "t"]:
        ext[nm] = nc.dram_tensor(nm, [T, H], F32, kind="ExternalInput")
    ext["em"] = nc.dram_tensor("em", [1, T], I32, kind="ExternalInput")
    ext["mk"] = nc.dram_tensor("mk", [1, T], I32, kind="ExternalInput")
    for nm in W_NAMES:
        ext[nm] = nc.dram_tensor(nm, [H, H], BF16, kind="ExternalInput")
    ext["ada_w"] = nc.dram_tensor("ada_w", [H, 2 * H], BF16, kind="ExternalInput")
    for nm in B_NAMES:
        d = 2 * H if nm == "ada_b" else H
        ext[nm] = nc.dram_tensor(nm, [1, d], BF16, kind="ExternalInput")
    for nm in LN_NAMES:
        ext[nm + "_g"] = nc.dram_tensor(nm + "_g", [1, H], F32, kind="ExternalInput")
        ext[nm + "_b"] = nc.dram_tensor(nm + "_b", [1, H], F32, kind="ExternalInput")
    ext["c_ones"] = nc.dram_tensor("c_ones", [1, T], BF16, kind="ExternalInput")
    ext["c_onesp"] = nc.dram_tensor("c_onesp", [128, NH], BF16, kind="ExternalInput")
    ext["c_ident"] = nc.dram_tensor("c_ident", [128, 128], F32, kind="ExternalInput")
    out_ext = nc.dram_tensor("out", [T, H], F32, kind="ExternalOutput")

    with tile.TileContext(nc) as tc:
        _emit(nc, tc, ext, out_ext, use_bias, use_affine)
    nc.compile()
    return nc


def _emit(nc, tc, ext, out_ext, use_bias, use_affine):
    import contextlib
    ctx = contextlib.ExitStack()
    with ctx:
        full = ctx.enter_context(tc.tile_pool(name="full", bufs=16))
        halfT = ctx.enter_context(tc.tile_pool(name="halfT", bufs=28))
        wpool = ctx.enter_context(tc.tile_pool(name="wpool", bufs=16))
        bmod = ctx.enter_context(tc.tile_pool(name="bmod", bufs=8))
        ktp = ctx.enter_context(tc.tile_pool(name="ktp", bufs=3))
        vtp = ctx.enter_context(tc.tile_pool(name="vtp", bufs=16))
        ppool = ctx.enter_context(tc.tile_pool(name="ppool", bufs=3))
        bcsp = ctx.enter_context(tc.tile_pool(name="bcsp", bufs=2))
        vaugp = ctx.enter_context(tc.tile_pool(name="vaugp", bufs=4))
        smalls = ctx.enter_context(tc.tile_pool(name="smalls", bufs=1))
        stat = ctx.enter_context(tc.tile_pool(name="stat", bufs=8))
        rowp = ctx.enter_context(tc.tile_pool(name="rowp", bufs=2))
        ps = ctx.enter_context(tc.tile_pool(name="ps", bufs=4, space="PSUM"))
        pspair = ctx.enter_context(tc.tile_pool(name="pspair", bufs=2, space="PSUM"))
        dram = ctx.enter_context(tc.tile_pool(name="dram", bufs=1, space="DRAM"))

        # ---------------- constants ----------------
        ones = smalls.tile([1, T], BF16, name="ones", tag="ones")
        nc.sync.dma_start(ones[:], ext["c_ones"].ap())
        onesp = smalls.tile([128, NH], BF16, name="onesp", tag="onesp")
        nc.sync.dma_start(onesp[:], ext["c_onesp"].ap())
        ident = smalls.tile([128, 128], F32, name="ident", tag="ident")
        nc.sync.dma_start(ident[:], ext["c_ident"].ap())
        eps_t = smalls.tile([128, 1], F32, name="eps_t", tag="eps_t")
        nc.vector.memset(eps_t[:], EPS)

        def load_row(nm, width=H, dt_=BF16):
            t_ = smalls.tile([1, width], dt_, name="row_" + nm, tag="row_" + nm)
            nc.sync.dma_start(t_[:], ext[nm].ap())
            return t_

        biases = {}
        if use_bias:
            for nm in B_NAMES:
                biases[nm] = load_row(nm, 2 * H if nm == "ada_b" else H)

        ln_par = {}
        if use_affine:
            for nm in LN_NAMES:
                bc = {}
                for which in ("g", "b"):
                    row = load_row(nm + "_" + which, H, F32)
                    rowb = smalls.tile([1, H], BF16, name=f"lnb_{nm}_{which}", tag=f"lnb_{nm}_{which}")
                    nc.vector.tensor_copy(rowb[:], row[:])
                    dst = smalls.tile([128, H], F32, name=f"ln_{nm}_{which}", tag=f"ln_{nm}_{which}")
                    for n in range(2):
                        p = ps.tile([128, 512], F32, name="ps", tag="ps")
                        nc.tensor.matmul(p[:], ones[:, 0:128],
                                         rowb[:, n * 512:(n + 1) * 512],
                                         start=True, stop=True)
                        nc.scalar.copy(dst[:, n * 512:(n + 1) * 512], p[:])
                    bc[which] = dst
                ln_par[nm] = bc

        def mask_bcast(name, tagn):
            mi = smalls.tile([1, T], I32, name=tagn + "_i", tag=tagn + "_i")
            nc.sync.dma_start(mi[:], ext[name].ap())
            mf = smalls.tile([1, T], F32, name=tagn + "_f", tag=tagn + "_f")
            nc.vector.tensor_copy(mf[:], mi[:])
            mr = smalls.tile([1, T], BF16, name=tagn + "_r", tag=tagn + "_r")
            nc.vector.tensor_scalar_mul(mr[:], mf[:], 0.125)
            bc = smalls.tile([128, T], BF16, name=tagn + "_bc", tag=tagn + "_bc")
            p = ps.tile([128, 512], F32, name="ps", tag="ps")
            nc.tensor.matmul(p[:], ones[:, 0:128], mr[:], start=True, stop=True)
            nc.vector.tensor_copy(bc[:], p[:])
            return bc

        em_bc = mask_bcast("em", "em")
        mk_bc = mask_bcast("mk", "mk")

        # ---------------- helpers ----------------
        def load_input(nm, tagn):
            tiles = []
            for mt in range(TC):
                t_ = full.tile([128, H], F32, name=tagn, tag="big")
                nc.sync.dma_start(t_[:], ext[nm].ap()[mt * 128:(mt + 1) * 128, :])
                tiles.append(t_)
            return tiles

        def load_weight(nm):
            tiles = []
            for k in range(HC):
                t_ = wpool.tile([128, H], BF16, name="w", tag="w")
                nc.sync.dma_start(t_[:], ext[nm].ap()[k * 128:(k + 1) * 128, :])
                tiles.append(t_)
            return tiles

        def layernorm(src_tiles, ln_name, out_tag):
            out_tiles = []
            for mt in range(TC):
                st = stat.tile([128, 12], F32, name="lnstat", tag="lnstat")
                nc.vector.bn_stats(st[:, 0:6], src_tiles[mt][:, 0:512])
                nc.vector.bn_stats(st[:, 6:12], src_tiles[mt][:, 512:1024])
                ag = stat.tile([128, 2], F32, name="lnag", tag="lnag")
                nc.vector.bn_aggr(ag[:], st[:])
                sd = stat.tile([128, 1], F32, name="lnsd", tag="lnsd")
                nc.scalar.activation(sd[:], ag[:, 1:2], AF.Sqrt, bias=eps_t[:])
                rstd = stat.tile([128, 1], F32, name="lnrstd", tag="lnrstd")
                nc.vector.reciprocal(rstd[:], sd[:])
                o = full.tile([128, H], F32, name=out_tag, tag="big")
                nc.vector.tensor_scalar(o[:], src_tiles[mt][:], ag[:, 0:1],
                                        rstd[:], op0=OP.subtract, op1=OP.mult)
                if use_affine and ln_name is not None:
                    nc.vector.tensor_mul(o[:], o[:], ln_par[ln_name]["g"][:])
                    nc.vector.tensor_add(o[:], o[:], ln_par[ln_name]["b"][:])
                out_tiles.append(o)
            return out_tiles

        def transpose_act(src_tiles, out_tag):
            """f32 natural [T,H] tiles -> 8 x bf16 [128, T] transposed tiles.
            mt-major order lets PE start on the first ready source tile."""
            out_tiles = []
            for hg in range(2):
                pts = [ps.tile([128, 512], F32, name="ps", tag="ps")
                       for _ in range(4)]
                for mt in range(TC):
                    for j in range(4):
                        hh = hg * 4 + j
                        nc.tensor.transpose(
                            pts[j][:, mt * 128:(mt + 1) * 128],
                            src_tiles[mt][:, hh * 128:(hh + 1) * 128], ident[:])
                for j in range(4):
                    o = halfT.tile([128, T], BF16, name=out_tag, tag="ht")
                    if j % 2 == 0:
                        nc.vector.tensor_copy(o[:], pts[j][:])
                    else:
                        nc.scalar.copy(o[:], pts[j][:])
                    out_tiles.append(o)
            return out_tiles

        def proj_T(w_tiles, actT_tiles, bias, out_tag, mo_list=None,
                   evict_mul=None):
            """out^T [H_out, T] = (act @ W)^T via lhsT=W chunk, rhs=act^T.
            evict_mul: optional [128, T] bf16 tile multiplied in during
            eviction (used to fold the query mask + 1/sqrt(dh) scale)."""
            out_tiles = []
            for mo in (mo_list if mo_list is not None else range(HC)):
                p = ps.tile([128, 512], F32, name="ps", tag="ps")
                for k in range(HC):
                    nc.tensor.matmul(
                        p[:], w_tiles[k][:, mo * 128:(mo + 1) * 128],
                        actT_tiles[k][:],
                        start=(k == 0), stop=(k == HC - 1 and bias is None))
                if bias is not None:
                    nc.tensor.matmul(p[:], bias[:, mo * 128:(mo + 1) * 128],
                                     ones[:], start=False, stop=True)
                o = halfT.tile([128, T], BF16, name=out_tag, tag="ht")
                if evict_mul is not None:
                    nc.vector.tensor_mul(o[:], p[:], evict_mul[:])
                elif mo % 2 == 0:
                    nc.scalar.copy(o[:], p[:])
                else:
                    nc.vector.tensor_copy(o[:], p[:])
                out_tiles.append(o)
            return out_tiles

        def proj_nat_psums(w_tiles, actT_tiles, bias, n_list=(0, 1)):
            for n in n_list:
                for mt in range(TC):
                    p = ps.tile([128, 512], F32, name="ps", tag="ps")
                    for k in range(HC):
                        nc.tensor.matmul(
                            p[:], actT_tiles[k][:, mt * 128:(mt + 1) * 128],
                            w_tiles[k][:, n * 512:(n + 1) * 512],
                            start=(k == 0), stop=(k == HC - 1 and bias is None))
                    if bias is not None:
                        nc.tensor.matmul(
                            p[:], ones[:, 0:128],
                            bias[:, n * 512:(n + 1) * 512],
                            start=False, stop=True)
                    yield mt, n, p

        def make_vaug_tiles():
            vaug_tiles = []
            for mt in range(TC):
                vt = vaugp.tile([128, VA_W], BF16, name="vt", tag="vaug")
                nc.vector.tensor_copy(vt[:, DH::DH + 1], onesp[:])
                vaug_tiles.append(vt)
            return vaug_tiles

        def proj_vaug(w_tiles, actT_tiles, bias, vaug_tiles, n_list=(0, 1)):
            """V projection evicted into padded bf16 [T, 1040] (+ones cols)."""
            for mt, n, p in proj_nat_psums(w_tiles, actT_tiles, bias, n_list):
                vt = vaug_tiles[mt]
                for j in range(8):
                    head = n * 8 + j
                    nc.vector.tensor_copy(
                        vt[:, head * (DH + 1):head * (DH + 1) + DH],
                        p[:, j * DH:(j + 1) * DH])
            return vaug_tiles

        def emit_kv(kT_tiles, vaug_tiles, ag_in):
            for hp in range(HC):
                nc.sync.dma_start(
                    ag_in[hp * (128 * T):(hp + 1) * (128 * T)]
                    .rearrange("(p f) -> p f", p=128),
                    kT_tiles[hp][:])
            for mt in range(TC):
                nc.sync.dma_start(
                    ag_in[KT_LEN + mt * (128 * VA_W):
                          KT_LEN + (mt + 1) * (128 * VA_W)]
                    .rearrange("(p f) -> p f", p=128),
                    vaug_tiles[mt][:])

        def emit_attention(qT_tiles, kt_src, vt_src, wo_tiles, bo,
                           resid_tiles, out_tag):
            """Software-pipelined attention over 64 flat iterations
            (hp-major: 8 head-pairs x 8 key-tile chunks).
            kt_src(hp, sl) -> DRAM AP [128, T]; vt_src(hpp, sl, ro) ->
            DRAM AP [128, 260] covering heads 4*hpp..4*hpp+3.
            qT tiles must already carry the mask * 1/sqrt(dh) factor.
            The per-head-pair normalization tail is staggered across the
            following iterations so its latency chain never head-of-line
            blocks the Vector/Scalar queues feeding the softmax exps."""
            NIT = HC * HC  # 64
            oT_tiles = [None] * HC
            kts = {}
            vts = {}
            accs = {}
            pairs = [None] * NIT
            pps = [None] * NIT
            tails = {}

            def load_kt(hp):
                kt = ktp.tile([128, 2 * T], BF16, name="kt", tag="kt")
                for sl in range(2):
                    nc.gpsimd.dma_start(kt[:, sl * T:(sl + 1) * T],
                                        kt_src(hp, sl))
                kts[hp] = kt

            def load_vts(hpp):
                lst = []
                for tk in range(HC):
                    sl, ro = tk // TC, (tk % TC) * 128
                    vt = vtp.tile([128, 4 * (DH + 1)], BF16, name="vt", tag="vt")
                    nc.gpsimd.dma_start(vt[:], vt_src(hpp, sl, ro))
                    lst.append(vt)
                vts[hpp] = lst

            def stage_scores(it):
                hp, tk = it // HC, it % HC
                if tk == 0:
                    if hp + 2 < HC:
                        load_kt(hp + 2)
                    accs[hp] = (ps.tile([128, 512], F32, name="oacc", tag="ps"),
                                ps.tile([128, 512], F32, name="oacc", tag="ps"))
                if tk == 4 and hp % 2 == 1 and hp // 2 + 1 < 4:
                    load_vts(hp // 2 + 1)
                pair = pspair.tile([128, 1024], F32, name="spair", tag="sp")
                kt = kts[hp]
                nc.tensor.matmul(pair[:, 0:512],
                                 kt[0:64, tk * 128:(tk + 1) * 128],
                                 qT_tiles[hp][0:64, :], start=True, stop=True,
                                 tile_position=(0, 0))
                nc.tensor.matmul(pair[:, 512:1024],
                                 kt[64:128, tk * 128:(tk + 1) * 128],
                                 qT_tiles[hp][64:128, :], start=True, stop=True,
                                 tile_position=(64, 0))
                pairs[it] = pair

            def stage_exp(it):
                pp = ppool.tile([128, 1024], BF16, name="pp", tag="pp")
                if DVE_EXP and (it % 8) in (2, 4, 6):
                    nc.vector.tensor_scalar(pp[:].bitcast(I16), pairs[it][:],
                                            EXP_A, EXP_B,
                                            op0=OP.mult, op1=OP.add)
                else:
                    nc.scalar.activation(pp[:], pairs[it][:], AF.Exp)
                pairs[it] = None
                pps[it] = pp

            def stage_pv(it, step):
                hp, tk = it // HC, it % HC
                hpp, i = hp // 2, hp % 2
                vt = vts[hpp][tk]
                pp = pps[it]
                oa, ob = accs[hp]
                nc.tensor.matmul(oa[0:DH + 1, :],
                                 vt[:, 2 * i * (DH + 1):(2 * i + 1) * (DH + 1)],
                                 pp[:, 0:512],
                                 start=(tk == 0), stop=(tk == HC - 1))
                nc.tensor.matmul(ob[0:DH + 1, :],
                                 vt[:, (2 * i + 1) * (DH + 1):(2 * i + 2) * (DH + 1)],
                                 pp[:, 512:1024],
                                 start=(tk == 0), stop=(tk == HC - 1))
                pps[it] = None
                if tk == HC - 1:
                    schedule_tail(hp, step)

            def schedule_tail(hp, step):
                oa, ob = accs.pop(hp)
                st = {}

                def t1():
                    st["den"] = rowp.tile([1, 2 * T], F32, name="den", tag="den")
                    nc.scalar.copy(st["den"][:, 0:T], oa[DH:DH + 1, :])
                    nc.scalar.copy(st["den"][:, T:2 * T], ob[DH:DH + 1, :])

                def t2():
                    st["recip"] = rowp.tile([1, 2 * T], F32, name="recip", tag="recip")
                    nc.vector.reciprocal_approx_fast(st["recip"][:], st["den"][:])

                def t25():
                    st["recr"] = rowp.tile([1, 2 * T], BF16, name="recr", tag="recr")
                    nc.scalar.copy(st["recr"][:], st["recip"][:])

                def t3():
                    st["bcs"] = bcsp.tile([64, 2 * T], BF16, name="bcs", tag="bcs")
                    nc.gpsimd.partition_broadcast(st["bcs"][:], st["recr"][:])

                def t4():
                    oT = halfT.tile([128, T], BF16, name="oT", tag="ht")
                    nc.vector.tensor_mul(oT[0:64, :], oa[0:64, :],
                                         st["bcs"][:, 0:T])
                    nc.vector.tensor_mul(oT[64:128, :], ob[0:64, :],
                                         st["bcs"][:, T:2 * T])
                    oT_tiles[hp] = oT

                for off, fn in ((1, t1), (2, t2), (3, t25), (4, t3), (5, t4)):
                    tails.setdefault(step + off, []).append(fn)

            load_kt(0)
            load_kt(1)
            load_vts(0)
            for step in range(NIT + 8):
                if step < NIT:
                    stage_scores(step)
                if 1 <= step <= NIT:
                    stage_exp(step - 1)
                if 2 <= step <= NIT + 1:
                    stage_pv(step - 2, step)
                for fn in tails.pop(step, ()):
                    fn()

            out_tiles = [full.tile([128, H], F32, name=out_tag, tag="big")
                         for _ in range(TC)]
            for mt in range(TC):
                for n in range(2):
                    p = ps.tile([128, 512], F32, name="ps", tag="ps")
                    for k in range(HC):
                        nc.tensor.matmul(
                            p[:], oT_tiles[k][:, mt * 128:(mt + 1) * 128],
                            wo_tiles[k][:, n * 512:(n + 1) * 512],
                            start=(k == 0), stop=(k == HC - 1 and bo is None))
                    if bo is not None:
                        nc.tensor.matmul(p[:], ones[:, 0:128],
                                         bo[:, n * 512:(n + 1) * 512],
                                         start=False, stop=True)
                    nc.vector.tensor_add(
                        out_tiles[mt][:, n * 512:(n + 1) * 512], p[:],
                        resid_tiles[mt][:, n * 512:(n + 1) * 512])
            return out_tiles

        # =====================================================================
        # Phase 1: adaLN chain -> hidden_in; self K/V; AllGather(self)
        # =====================================================================
        h_sb, t_sb = [], []
        for mt in range(TC):
            th = full.tile([128, H], F32, name="h", tag="big")
            nc.sync.dma_start(th[:], ext["h"].ap()[mt * 128:(mt + 1) * 128, :])
            h_sb.append(th)
            tt = full.tile([128, H], F32, name="tin", tag="big")
            nc.sync.dma_start(tt[:], ext["t"].ap()[mt * 128:(mt + 1) * 128, :])
            t_sb.append(tt)
        x_sb = load_input("x", "x")

        ag_in_s0 = dram.tile([HALF_LEN], BF16, name="agins0", tag="agins0")
        ag_out_s0 = dram.tile([2 * HALF_LEN], BF16, name="agouts0", tag="agouts0")
        ag_in_s1 = dram.tile([HALF_LEN], BF16, name="agins1", tag="agins1")
        ag_out_s1 = dram.tile([2 * HALF_LEN], BF16, name="agouts1", tag="agouts1")
        ag_in_c = dram.tile([RANK_LEN], BF16, name="aginc", tag="aginc")
        ag_out_c = dram.tile([2 * RANK_LEN], BF16, name="agoutc", tag="agoutc")

        with nc.named_scope("p1_ada"):
            silu = []
            for mt in range(TC):
                nc.vector.tensor_add(t_sb[mt][:], t_sb[mt][:], h_sb[mt][:])
                nc.scalar.activation(t_sb[mt][:], t_sb[mt][:], AF.Silu)
                silu.append(t_sb[mt])
            siluT = transpose_act(silu, "siluT")
            # hn only needs h: compute early to fill PE while ada_w DMA lands
            hn = layernorm(h_sb, "lnh", "hn")
            hnT = transpose_act(hn, "hnT")

            shift_sb = [bmod.tile([128, H], BF16, name="shift", tag="mod") for _ in range(TC)]
            scale1_sb = [bmod.tile([128, H], BF16, name="scale1", tag="mod") for _ in range(TC)]
            for half in range(2):
                ada_tiles = []
                for k in range(HC):
                    t_ = wpool.tile([128, H], BF16, name="ada", tag="w")
                    nc.sync.dma_start(
                        t_[:], ext["ada_w"].ap()[k * 128:(k + 1) * 128,
                                                 half * H:(half + 1) * H])
                    ada_tiles.append(t_)
                for mt in range(TC):
                    for hn_ in range(2):
                        p = ps.tile([128, 512], F32, name="ps", tag="ps")
                        for k in range(HC):
                            nc.tensor.matmul(
                                p[:], siluT[k][:, mt * 128:(mt + 1) * 128],
                                ada_tiles[k][:, hn_ * 512:(hn_ + 1) * 512],
                                start=(k == 0), stop=(k == HC - 1 and not use_bias))
                        if use_bias:
                            nc.tensor.matmul(
                                p[:], ones[:, 0:128],
                                biases["ada_b"][:, (half * 2 + hn_) * 512:
                                                (half * 2 + hn_ + 1) * 512],
                                start=False, stop=True)
                        dst = shift_sb[mt] if half == 0 else scale1_sb[mt]
                        nc.scalar.activation(dst[:, hn_ * 512:(hn_ + 1) * 512], p[:],
                                             AF.Copy, bias=(1.0 if half == 1 else 0.0))

            xln = layernorm(x_sb, None, "xln")
            hin = []
            for mt in range(TC):
                nc.vector.tensor_mul(xln[mt][:], xln[mt][:], scale1_sb[mt][:])
                nc.vector.tensor_add(xln[mt][:], xln[mt][:], shift_sb[mt][:])
                hin.append(xln[mt])
            hinT = transpose_act(hin, "hinT")

        with nc.named_scope("p2_selfkv"):
            wsk = load_weight("Wsk")
            wsv = load_weight("Wsv")
            vaug_s = make_vaug_tiles()
            ksT = [None] * HC
            ag_s = (ag_in_s0, ag_in_s1)
            ag_so = (ag_out_s0, ag_out_s1)
            for halfk in range(2):
                part = proj_T(wsk, hinT, biases.get("bsk"), "ksT",
                              mo_list=range(4 * halfk, 4 * halfk + 4))
                for j, mo in enumerate(range(4 * halfk, 4 * halfk + 4)):
                    ksT[mo] = part[j]
                proj_vaug(wsv, hinT, biases.get("bsv"), vaug_s, n_list=(halfk,))
                agi = ag_s[halfk]
                for j, mo in enumerate(range(4 * halfk, 4 * halfk + 4)):
                    nc.sync.dma_start(
                        agi[j * (128 * T):(j + 1) * (128 * T)]
                        .rearrange("(p f) -> p f", p=128), ksT[mo][:])
                for mt in range(TC):
                    nc.sync.dma_start(
                        agi[KT_H + mt * (128 * (VA_W // 2)):
                            KT_H + (mt + 1) * (128 * (VA_W // 2))]
                        .rearrange("(p f) -> p f", p=128),
                        vaug_s[mt][:, halfk * (VA_W // 2):(halfk + 1) * (VA_W // 2)])
                nc.gpsimd.collective_compute(
                    "AllGather", OP.bypass,
                    replica_groups=[[0, 1], [2, 3], [4, 5], [6, 7]],
                    ins=[agi.opt()], outs=[ag_so[halfk].opt()])

            wsq = load_weight("Wsq")
            qsT = proj_T(wsq, hinT, biases.get("bsq"), "qsT", evict_mul=em_bc)

        # =====================================================================
        # Phase 2: cross K/V from h_n = lnh(h); AllGather(cross)
        # =====================================================================
        with nc.named_scope("p3_crosskv"):
            wck = load_weight("Wck")
            kcT = proj_T(wck, hnT, biases.get("bck"), "kcT")
            wcv = load_weight("Wcv")
            vaug_c = make_vaug_tiles()
            proj_vaug(wcv, hnT, biases.get("bcv"), vaug_c)
            emit_kv(kcT, vaug_c, ag_in_c)
            nc.gpsimd.collective_compute(
                "AllGather", OP.bypass,
                replica_groups=[[0, 1], [2, 3], [4, 5], [6, 7]],
                ins=[ag_in_c.opt()], outs=[ag_out_c.opt()])

        def kt_src_s(hp, sl):
            half, hpl = hp // 4, hp % 4
            return ag_so[half][sl * HALF_LEN + hpl * (128 * T):
                               sl * HALF_LEN + (hpl + 1) * (128 * T)] \
                .rearrange("(p f) -> p f", p=128)

        def vt_src_s(hpp, sl, ro):
            half, hl = hpp // 2, hpp % 2
            v = ag_so[half][sl * HALF_LEN + KT_H:
                            sl * HALF_LEN + KT_H + VA_H] \
                .rearrange("(tt f) -> tt f", tt=T)
            return v[ro:ro + 128, hl * 260:(hl + 1) * 260]

        def kt_src_c(hp, sl):
            return ag_out_c[sl * RANK_LEN + hp * (128 * T):
                            sl * RANK_LEN + (hp + 1) * (128 * T)] \
                .rearrange("(p f) -> p f", p=128)

        def vt_src_c(hpp, sl, ro):
            v = ag_out_c[sl * RANK_LEN + KT_LEN:
                         sl * RANK_LEN + KT_LEN + VA_LEN] \
                .rearrange("(tt f) -> tt f", tt=T)
            return v[ro:ro + 128, hpp * 260:(hpp + 1) * 260]

        # =====================================================================
        # Phase 3: self attention -> hidden_in(+x); cross q
        # =====================================================================
        with nc.named_scope("p4_selfattn"):
            wso = load_weight("Wso")
            hidden_in = emit_attention(qsT, kt_src_s, vt_src_s, wso,
                                       biases.get("bso"), x_sb, "res1")

        with nc.named_scope("p5_ln2q"):
            ln2o = layernorm(hidden_in, "ln2", "ln2o")
            ln2T = transpose_act(ln2o, "ln2T")
            wcq = load_weight("Wcq")
            qcT = proj_T(wcq, ln2T, biases.get("bcq"), "qcT", evict_mul=mk_bc)

        # =====================================================================
        # Phase 4: cross attention -> hidden_mid
        # =====================================================================
        with nc.named_scope("p6_crossattn"):
            wco = load_weight("Wco")
            hidden_mid = emit_attention(qcT, kt_src_c, vt_src_c, wco,
                                        biases.get("bco"), hidden_in, "hmid")

        # =====================================================================
        # Phase 5: FFN + final LN + output
        # =====================================================================
        with nc.named_scope("p7_ffn"):
            ln3o = layernorm(hidden_mid, "ln3", "ln3o")
            hoT = transpose_act(ln3o, "hoT")
            w1 = load_weight("ffn_w1")
            midT = []
            for mo in range(HC):
                p = ps.tile([128, 512], F32, name="ps", tag="ps")
                for k in range(HC):
                    nc.tensor.matmul(p[:], w1[k][:, mo * 128:(mo + 1) * 128],
                                     hoT[k][:], start=(k == 0),
                                     stop=(k == HC - 1 and not use_bias))
                if use_bias:
                    nc.tensor.matmul(
                        p[:], biases["ffn_b1"][:, mo * 128:(mo + 1) * 128],
                        ones[:], start=False, stop=True)
                o = halfT.tile([128, T], BF16, name="midT", tag="ht")
                if mo % 2 == 0:
                    nc.scalar.activation(o[:], p[:], AF.Relu)
                else:
                    nc.vector.tensor_scalar_max(o[:], p[:], 0.0)
                midT.append(o)
            w2 = load_weight("ffn_w2")
            ffres = [full.tile([128, H], F32, name="ffres", tag="big") for _ in range(TC)]
            for mt, n, p in proj_nat_psums(w2, midT, biases.get("ffn_b2")):
                nc.vector.tensor_add(ffres[mt][:, n * 512:(n + 1) * 512], p[:],
                                     ln3o[mt][:, n * 512:(n + 1) * 512])

            lnfo = layernorm(ffres, "lnf", "lnfo")
            for mt in range(TC):
                nc.vector.tensor_add(lnfo[mt][:], lnfo[mt][:], hidden_mid[mt][:])
                nc.sync.dma_start(out_ext.ap()[mt * 128:(mt + 1) * 128, :],
                                  lnfo[mt][:])


def _bf16(a):
    import ml_dtypes
    return np.asarray(a, np.float32).astype(ml_dtypes.bfloat16)


def kernel(**inputs):
    global LAST_RESULT
    use_bias = any(np.any(np.asarray(inputs[nm])) for nm in B_NAMES)
    use_affine = any(
        (not np.array_equal(np.asarray(inputs[nm + "_g"]),
                            np.ones_like(np.asarray(inputs[nm + "_g"])))) or
        np.any(np.asarray(inputs[nm + "_b"]))
        for nm in LN_NAMES)
    flags = (use_bias, use_affine)
    if flags not in _BUILD_CACHE:
        _BUILD_CACHE[flags] = _build(flags)
    nc = _BUILD_CACHE[flags]

    x = np.asarray(inputs["x"], np.float32)
    h = np.asarray(inputs["h"], np.float32)
    t = np.asarray(inputs["t"], np.float32)
    em = np.asarray(inputs["extent_mask"], np.int32)
    mk = np.asarray(inputs["mask"], np.int32)

    common = {}
    for nm in W_NAMES + ["ada_w"]:
        common[nm] = np.ascontiguousarray(_bf16(inputs[nm]))
    for nm in B_NAMES:
        common[nm] = np.ascontiguousarray(_bf16(inputs[nm]).reshape(1, -1))
    for nm in LN_NAMES:
        common[nm + "_g"] = np.asarray(inputs[nm + "_g"], np.float32).reshape(1, -1)
        common[nm + "_b"] = np.asarray(inputs[nm + "_b"], np.float32).reshape(1, -1)
    common["c_ones"] = _bf16(np.ones((1, T)))
    common["c_onesp"] = _bf16(np.ones((128, NH)))
    common["c_ident"] = np.eye(128, dtype=np.float32)

    in_maps = []
    for c in range(NC):
        b, half = c // 2, c % 2
        s0 = half * T
        m = dict(common)
        m["x"] = np.ascontiguousarray(x[b, s0:s0 + T])
        m["h"] = np.ascontiguousarray(h[b, s0:s0 + T])
        m["t"] = np.ascontiguousarray(t[b, s0:s0 + T])
        m["em"] = np.ascontiguousarray(em[b, s0:s0 + T].reshape(1, T))
        m["mk"] = np.ascontiguousarray(mk[b, s0:s0 + T].reshape(1, T))
        in_maps.append(m)

    trace = bool(os.environ.get("BASS_TRACE_KERNEL"))
    if trace:
        _install_ntff_hook()
    try:
        res = bass_utils.run_bass_kernel_spmd(
            nc, in_maps, core_ids=list(range(NC)), trace=trace)
    except Exception:
        import time
        time.sleep(20)
        res = bass_utils.run_bass_kernel_spmd(
            nc, in_maps, core_ids=list(range(NC)), trace=trace)
    LAST_RESULT = res

    out = np.empty((B, S, H), np.float32)
    for c in range(NC):
        b, half = c // 2, c % 2
        out[b, half * T:(half + 1) * T] = res.results[c]["out"]
    return out


def _install_ntff_hook():
    import sys, types
    if 'antenv.axon_hooks' in sys.modules:
        return
    mod = types.ModuleType("antenv.axon_hooks")
    mod._hook = None
    def set_axon_ntff_profile_hook(h): mod._hook = h
    def get_axon_ntff_profile_hook(): return mod._hook
    mod.set_axon_ntff_profile_hook = set_axon_ntff_profile_hook
    mod.get_axon_ntff_profile_hook = get_axon_ntff_profile_hook
    sys.modules['antenv.axon_hooks'] = mod
    import antenv
    antenv.axon_hooks = mod
    try:
        from trn_agent_boot.trn_boot import _ntff_profile_via_ctypes
        mod.set_axon_ntff_profile_hook(
            _ntff_profile_via_ctypes('/opt/axon/libaxon_pjrt.so'))
    except Exception:
        pass


# revision 11
# speedup vs baseline: 1.0922x; 1.0922x over previous
"""AdaLN transformer block on 8 TRN2 NeuronCores (Bass/Tile).

Sharding: 4096 tokens (B*S) split 8 ways -> 512 tokens/core; cores (2b, 2b+1)
own batch element b. All per-token compute (LN, adaLN modulation, projections,
FFN) is perfectly sharded, zero duplication. Attention needs full-sequence K/V
per batch element: each core computes K^T / V(+ones col) for its own tokens in
bf16 and a pairwise AllGather (replica groups [[0,1],[2,3],[4,5],[6,7]])
replicates them within each batch pair. The gather output layout is
core-independent: slot 0 = the batch's first 512 tokens, slot 1 = second half.

Precision: all matmuls bf16 (weights pre-cast to bf16 on the host), fp32
accumulation in PSUM. The f32 spine (inputs, LN stats, residuals, softmax
denominators) keeps the error small.

Attention is software-pipelined over a flat 64-iteration space (8 head-pairs x
8 key-tiles): scores(it) -> exp(it-1) -> PV(it-2), so the in-order PE queue
never stalls on the scores->exp->PV dependency chain. Each iteration's two
score matmuls land in one 2-bank PSUM tile [128,1024]; softmax exp is one op
per iteration, alternating between ScalarE (AF.Exp) and VectorE (Schraudolph
bit-trick: int16(x*184.665+16249) reinterpreted as bf16). Query-row masking
(+the 1/sqrt(dh) scale) is folded into the q projection eviction. Softmax
denominators come from an appended ones-column in V (M=65 PV matmuls).
"""
import os
import numpy as np

import concourse.bass as bass
import concourse.bacc as bacc
import concourse.tile as tile
import concourse.mybir as mybir
from concourse import bass_utils

B, S, H, NH = 4, 1024, 1024, 16
DH = H // NH               # 64
EPS = 1e-5
NC = 8
T = (B * S) // NC          # 512 tokens per core
TC = T // 128              # 4
HC = H // 128              # 8
VA_W = NH * (DH + 1)       # 1040
KT_LEN = H * T             # 524288 elements
KT_H = (H // 2) * T        # 262144: half the heads' k^T
VA_H = T * (VA_W // 2)     # 266240: half the heads' v_aug
HALF_LEN = KT_H + VA_H
VA_LEN = T * VA_W          # 532480 elements
RANK_LEN = KT_LEN + VA_LEN

F32 = mybir.dt.float32
BF16 = mybir.dt.bfloat16
I16 = mybir.dt.int16
I32 = mybir.dt.int32
AF = mybir.ActivationFunctionType
OP = mybir.AluOpType

# Schraudolph fast-exp constants for bf16 bit-pattern output (int16 trunc).
EXP_A = 184.6650390625     # 2^7 / ln(2)
EXP_B = 16249.0            # 127*2^7 - c (c tuned, incl. +0.5 trunc comp)
DVE_EXP = True             # half of softmax exps on VectorE via bit-trick

W_NAMES = ["Wsq", "Wsk", "Wsv", "Wso", "Wcq", "Wck", "Wcv", "Wco",
           "ffn_w1", "ffn_w2"]
B_NAMES = ["bsq", "bsk", "bsv", "bso", "bcq", "bck", "bcv", "bco",
           "ffn_b1", "ffn_b2", "ada_b"]
LN_NAMES = ["ln2", "ln3", "lnh", "lnf"]

LAST_RESULT = None
_BUILD_CACHE = {}


def _build(flags):
    use_bias, use_affine = flags
    nc = bacc.Bacc("TRN2", target_bir_lowering=False, debug=False,
                   num_devices=NC)
    ext = {}
    for nm in ["x", "h", "t"]:
        ext[nm] = nc.dram_tensor(nm, [T, H], F32, kind="ExternalInput")
    ext["em"] = nc.dram_tensor("em", [1, T], I32, kind="ExternalInput")
    ext["mk"] = nc.dram_tensor("mk", [1, T], I32, kind="ExternalInput")
    for nm in W_NAMES:
        ext[nm] = nc.dram_tensor(nm, [H, H], BF16, kind="ExternalInput")
    ext["ada_w"] = nc.dram_tensor("ada_w", [H, 2 * H], BF16, kind="ExternalInput")
    for nm in B_NAMES:
        d = 2 * H if nm == "ada_b" else H
        ext[nm] = nc.dram_tensor(nm, [1, d], BF16, kind="ExternalInput")
    for nm in LN_NAMES:
        ext[nm + "_g"] = nc.dram_tensor(nm + "_g", [1, H], F32, kind="ExternalInput")
        ext[nm + "_b"] = nc.dram_tensor(nm + "_b", [1, H], F32, kind="ExternalInput")
    ext["c_ones"] = nc.dram_tensor("c_ones", [1, T], BF16, kind="ExternalInput")
    ext["c_onesp"] = nc.dram_tensor("c_onesp", [128, NH], BF16, kind="ExternalInput")
    ext["c_ident"] = nc.dram_tensor("c_ident", [128, 128], F32, kind="ExternalInput")
    out_ext = nc.dram_tensor("out", [T, H], F32, kind="ExternalOutput")

    with tile.TileContext(nc) as tc:
        _emit(nc, tc, ext, out_ext, use_bias, use_affine)
    nc.compile()
    return nc


def _emit(nc, tc, ext, out_ext, use_bias, use_affine):
    import contextlib
    ctx = contextlib.ExitStack()
    with ctx:
        full = ctx.enter_context(tc.tile_pool(name="full", bufs=16))
        halfT = ctx.enter_context(tc.tile_pool(name="halfT", bufs=28))
        wpool = ctx.enter_context(tc.tile_pool(name="wpool", bufs=16))
        bmod = ctx.enter_context(tc.tile_pool(name="bmod", bufs=8))
        ktp = ctx.enter_context(tc.tile_pool(name="ktp", bufs=3))
        vtp = ctx.enter_context(tc.tile_pool(name="vtp", bufs=16))
        ppool = ctx.enter_context(tc.tile_pool(name="ppool", bufs=3))
        bcsp = ctx.enter_context(tc.tile_pool(name="bcsp", bufs=2))
        vaugp = ctx.enter_context(tc.tile_pool(name="vaugp", bufs=4))
        smalls = ctx.enter_context(tc.tile_pool(name="smalls", bufs=1))
        stat = ctx.enter_context(tc.tile_pool(name="stat", bufs=8))
        rowp = ctx.enter_context(tc.tile_pool(name="rowp", bufs=2))
        ps = ctx.enter_context(tc.tile_pool(name="ps", bufs=4, space="PSUM"))
        pspair = ctx.enter_context(tc.tile_pool(name="pspair", bufs=2, space="PSUM"))
        dram = ctx.enter_context(tc.tile_pool(name="dram", bufs=1, space="DRAM"))

        # ---------------- constants ----------------
        ones = smalls.tile([1, T], BF16, name="ones", tag="ones")
        nc.sync.dma_start(ones[:], ext["c_ones"].ap())
        onesp = smalls.tile([128, NH], BF16, name="onesp", tag="onesp")
        nc.sync.dma_start(onesp[:], ext["c_onesp"].ap())
        ident = smalls.tile([128, 128], F32, name="ident", tag="ident")
        nc.sync.dma_start(ident[:], ext["c_ident"].ap())
        eps_t = smalls.tile([128, 1], F32, name="eps_t", tag="eps_t")
        nc.vector.memset(eps_t[:], EPS)

        def load_row(nm, width=H, dt_=BF16):
            t_ = smalls.tile([1, width], dt_, name="row_" + nm, tag="row_" + nm)
            nc.sync.dma_start(t_[:], ext[nm].ap())
            return t_

        biases = {}
        if use_bias:
            for nm in B_NAMES:
                biases[nm] = load_row(nm, 2 * H if nm == "ada_b" else H)

        ln_par = {}
        if use_affine:
            for nm in LN_NAMES:
                bc = {}
                for which in ("g", "b"):
                    row = load_row(nm + "_" + which, H, F32)
                    rowb = smalls.tile([1, H], BF16, name=f"lnb_{nm}_{which}", tag=f"lnb_{nm}_{which}")
                    nc.vector.tensor_copy(rowb[:], row[:])
                    dst = smalls.tile([128, H], F32, name=f"ln_{nm}_{which}", tag=f"ln_{nm}_{which}")
                    for n in range(2):
                        p = ps.tile([128, 512], F32, name="ps", tag="ps")
                        nc.tensor.matmul(p[:], ones[:, 0:128],
                                         rowb[:, n * 512:(n + 1) * 512],
                                         start=True, stop=True)
                        nc.scalar.copy(dst[:, n * 512:(n + 1) * 512], p[:])
                    bc[which] = dst
                ln_par[nm] = bc

        def mask_bcast(name, tagn):
            mi = smalls.tile([1, T], I32, name=tagn + "_i", tag=tagn + "_i")
            nc.sync.dma_start(mi[:], ext[name].ap())
            mf = smalls.tile([1, T], F32, name=tagn + "_f", tag=tagn + "_f")
            nc.vector.tensor_copy(mf[:], mi[:])
            mr = smalls.tile([1, T], BF16, name=tagn + "_r", tag=tagn + "_r")
            nc.vector.tensor_scalar_mul(mr[:], mf[:], 0.125)
            bc = smalls.tile([128, T], BF16, name=tagn + "_bc", tag=tagn + "_bc")
            p = ps.tile([128, 512], F32, name="ps", tag="ps")
            nc.tensor.matmul(p[:], ones[:, 0:128], mr[:], start=True, stop=True)
            nc.vector.tensor_copy(bc[:], p[:])
            return bc

        em_bc = mask_bcast("em", "em")
        mk_bc = mask_bcast("mk", "mk")

        # ---------------- helpers ----------------
        def load_input(nm, tagn):
            tiles = []
            for mt in range(TC):
                t_ = full.tile([128, H], F32, name=tagn, tag="big")
                nc.sync.dma_start(t_[:], ext[nm].ap()[mt * 128:(mt + 1) * 128, :])
                tiles.append(t_)
            return tiles

        def load_weight(nm):
            tiles = []
            for k in range(HC):
                t_ = wpool.tile([128, H], BF16, name="w", tag="w")
                nc.sync.dma_start(t_[:], ext[nm].ap()[k * 128:(k + 1) * 128, :])
                tiles.append(t_)
            return tiles

        def layernorm(src_tiles, ln_name, out_tag):
            out_tiles = []
            for mt in range(TC):
                st = stat.tile([128, 12], F32, name="lnstat", tag="lnstat")
                nc.vector.bn_stats(st[:, 0:6], src_tiles[mt][:, 0:512])
                nc.vector.bn_stats(st[:, 6:12], src_tiles[mt][:, 512:1024])
                ag = stat.tile([128, 2], F32, name="lnag", tag="lnag")
                nc.vector.bn_aggr(ag[:], st[:])
                sd = stat.tile([128, 1], F32, name="lnsd", tag="lnsd")
                nc.scalar.activation(sd[:], ag[:, 1:2], AF.Sqrt, bias=eps_t[:])
                rstd = stat.tile([128, 1], F32, name="lnrstd", tag="lnrstd")
                nc.vector.reciprocal(rstd[:], sd[:])
                o = full.tile([128, H], F32, name=out_tag, tag="big")
                nc.vector.tensor_scalar(o[:], src_tiles[mt][:], ag[:, 0:1],
                                        rstd[:], op0=OP.subtract, op1=OP.mult)
                if use_affine and ln_name is not None:
                    nc.vector.tensor_mul(o[:], o[:], ln_par[ln_name]["g"][:])
                    nc.vector.tensor_add(o[:], o[:], ln_par[ln_name]["b"][:])
                out_tiles.append(o)
            return out_tiles

        def transpose_act(src_tiles, out_tag):
            """f32 natural [T,H] tiles -> 8 x bf16 [128, T] transposed tiles.
            mt-major order lets PE start on the first ready source tile."""
            out_tiles = []
            for hg in range(2):
                pts = [ps.tile([128, 512], F32, name="ps", tag="ps")
                       for _ in range(4)]
                for mt in range(TC):
                    for j in range(4):
                        hh = hg * 4 + j
                        nc.tensor.transpose(
                            pts[j][:, mt * 128:(mt + 1) * 128],
                            src_tiles[mt][:, hh * 128:(hh + 1) * 128], ident[:])
                for j in range(4):
                    o = halfT.tile([128, T], BF16, name=out_tag, tag="ht")
                    if j % 2 == 0:
                        nc.vector.tensor_copy(o[:], pts[j][:])
                    else:
                        nc.scalar.copy(o[:], pts[j][:])
                    out_tiles.append(o)
            return out_tiles

        def proj_T(w_tiles, actT_tiles, bias, out_tag, mo_list=None,
                   evict_mul=None):
            """out^T [H_out, T] = (act @ W)^T via lhsT=W chunk, rhs=act^T.
            evict_mul: optional [128, T] bf16 tile multiplied in during
            eviction (used to fold the query mask + 1/sqrt(dh) scale)."""
            out_tiles = []
            for mo in (mo_list if mo_list is not None else range(HC)):
                p = ps.tile([128, 512], F32, name="ps", tag="ps")
                for k in range(HC):
                    nc.tensor.matmul(
                        p[:], w_tiles[k][:, mo * 128:(mo + 1) * 128],
                        actT_tiles[k][:],
                        start=(k == 0), stop=(k == HC - 1 and bias is None))
                if bias is not None:
                    nc.tensor.matmul(p[:], bias[:, mo * 128:(mo + 1) * 128],
                                     ones[:], start=False, stop=True)
                o = halfT.tile([128, T], BF16, name=out_tag, tag="ht")
                if evict_mul is not None:
                    nc.vector.tensor_mul(o[:], p[:], evict_mul[:])
                elif mo % 2 == 0:
                    nc.scalar.copy(o[:], p[:])
                else:
                    nc.vector.tensor_copy(o[:], p[:])
                out_tiles.append(o)
            return out_tiles

        def proj_nat_psums(w_tiles, actT_tiles, bias, n_list=(0, 1)):
            for n in n_list:
                for mt in range(TC):
                    p = ps.tile([128, 512], F32, name="ps", tag="ps")
                    for k in range(HC):
                        nc.tensor.matmul(
                            p[:], actT_tiles[k][:, mt * 128:(mt + 1) * 128],
                            w_tiles[k][:, n * 512:(n + 1) * 512],
                            start=(k == 0), stop=(k == HC - 1 and bias is None))
                    if bias is not None:
                        nc.tensor.matmul(
                            p[:], ones[:, 0:128],
                            bias[:, n * 512:(n + 1) * 512],
                            start=False, stop=True)
                    yield mt, n, p

        def make_vaug_tiles():
            vaug_tiles = []
            for mt in range(TC):
                vt = vaugp.tile([128, VA_W], BF16, name="vt", tag="vaug")
                nc.vector.tensor_copy(vt[:, DH::DH + 1], onesp[:])
                vaug_tiles.append(vt)
            return vaug_tiles

        def proj_vaug(w_tiles, actT_tiles, bias, vaug_tiles, n_list=(0, 1)):
            """V projection evicted into padded bf16 [T, 1040] (+ones cols)."""
            for mt, n, p in proj_nat_psums(w_tiles, actT_tiles, bias, n_list):
                vt = vaug_tiles[mt]
                for j in range(8):
                    head = n * 8 + j
                    nc.vector.tensor_copy(
                        vt[:, head * (DH + 1):head * (DH + 1) + DH],
                        p[:, j * DH:(j + 1) * DH])
            return vaug_tiles

        def emit_kv(kT_tiles, vaug_tiles, ag_in):
            for hp in range(HC):
                nc.sync.dma_start(
                    ag_in[hp * (128 * T):(hp + 1) * (128 * T)]
                    .rearrange("(p f) -> p f", p=128),
                    kT_tiles[hp][:])
            for mt in range(TC):
                nc.sync.dma_start(
                    ag_in[KT_LEN + mt * (128 * VA_W):
                          KT_LEN + (mt + 1) * (128 * VA_W)]
                    .rearrange("(p f) -> p f", p=128),
                    vaug_tiles[mt][:])

        def emit_attention(qT_tiles, kt_src, vt_src, wo_tiles, bo,
                           resid_tiles, out_tag):
            """Software-pipelined attention over 64 flat iterations
            (hp-major: 8 head-pairs x 8 key-tile chunks).
            kt_src(hp, sl) -> DRAM AP [128, T]; vt_src(hpp, sl, ro) ->
            DRAM AP [128, 260] covering heads 4*hpp..4*hpp+3.
            qT tiles must already carry the mask * 1/sqrt(dh) factor.
            The per-head-pair normalization tail is staggered across the
            following iterations so its latency chain never head-of-line
            blocks the Vector/Scalar queues feeding the softmax exps."""
            NIT = HC * HC  # 64
            oT_tiles = [None] * HC
            kts = {}
            vts = {}
            accs = {}
            pairs = [None] * NIT
            pps = [None] * NIT
            tails = {}

            def load_kt(hp):
                kt = ktp.tile([128, 2 * T], BF16, name="kt", tag="kt")
                for sl in range(2):
                    nc.sync.dma_start(kt[:, sl * T:(sl + 1) * T],
                                      kt_src(hp, sl))
                kts[hp] = kt

            def load_vts(hpp):
                lst = []
                for tk in range(HC):
                    sl, ro = tk // TC, (tk % TC) * 128
                    vt = vtp.tile([128, 4 * (DH + 1)], BF16, name="vt", tag="vt")
                    nc.sync.dma_start(vt[:], vt_src(hpp, sl, ro))
                    lst.append(vt)
                vts[hpp] = lst

            def stage_scores(it):
                hp, tk = it // HC, it % HC
                if tk == 0:
                    if hp + 2 < HC:
                        load_kt(hp + 2)
                    accs[hp] = (ps.tile([128, 512], F32, name="oacc", tag="ps"),
                                ps.tile([128, 512], F32, name="oacc", tag="ps"))
                if tk == 4 and hp % 2 == 1 and hp // 2 + 1 < 4:
                    load_vts(hp // 2 + 1)
                pair = pspair.tile([128, 1024], F32, name="spair", tag="sp")
                kt = kts[hp]
                nc.tensor.matmul(pair[:, 0:512],
                                 kt[0:64, tk * 128:(tk + 1) * 128],
                                 qT_tiles[hp][0:64, :], start=True, stop=True,
                                 tile_position=(0, 0))
                nc.tensor.matmul(pair[:, 512:1024],
                                 kt[64:128, tk * 128:(tk + 1) * 128],
                                 qT_tiles[hp][64:128, :], start=True, stop=True,
                                 tile_position=(64, 0))
                pairs[it] = pair

            def stage_exp(it):
                pp = ppool.tile([128, 1024], BF16, name="pp", tag="pp")
                if DVE_EXP and (it % 8) in (2, 4, 6):
                    nc.vector.tensor_scalar(pp[:].bitcast(I16), pairs[it][:],
                                            EXP_A, EXP_B,
                                            op0=OP.mult, op1=OP.add)
                else:
                    nc.scalar.activation(pp[:], pairs[it][:], AF.Exp)
                pairs[it] = None
                pps[it] = pp

            def stage_pv(it, step):
                hp, tk = it // HC, it % HC
                hpp, i = hp // 2, hp % 2
                vt = vts[hpp][tk]
                pp = pps[it]
                oa, ob = accs[hp]
                nc.tensor.matmul(oa[0:DH + 1, :],
                                 vt[:, 2 * i * (DH + 1):(2 * i + 1) * (DH + 1)],
                                 pp[:, 0:512],
                                 start=(tk == 0), stop=(tk == HC - 1))
                nc.tensor.matmul(ob[0:DH + 1, :],
                                 vt[:, (2 * i + 1) * (DH + 1):(2 * i + 2) * (DH + 1)],
                                 pp[:, 512:1024],
                                 start=(tk == 0), stop=(tk == HC - 1))
                pps[it] = None
                if tk == HC - 1:
                    schedule_tail(hp, step)

            def schedule_tail(hp, step):
                oa, ob = accs.pop(hp)
                st = {}

                def t1():
                    st["den"] = rowp.tile([1, 2 * T], F32, name="den", tag="den")
                    nc.scalar.copy(st["den"][:, 0:T], oa[DH:DH + 1, :])
                    nc.scalar.copy(st["den"][:, T:2 * T], ob[DH:DH + 1, :])

                def t2():
                    st["recip"] = rowp.tile([1, 2 * T], F32, name="recip", tag="recip")
                    nc.vector.reciprocal_approx_fast(st["recip"][:], st["den"][:])

                def t25():
                    st["recr"] = rowp.tile([1, 2 * T], BF16, name="recr", tag="recr")
                    nc.scalar.copy(st["recr"][:], st["recip"][:])

                def t3():
                    st["bcs"] = bcsp.tile([64, 2 * T], BF16, name="bcs", tag="bcs")
                    nc.gpsimd.partition_broadcast(st["bcs"][:], st["recr"][:])

                def t4():
                    oT = halfT.tile([128, T], BF16, name="oT", tag="ht")
                    nc.vector.tensor_mul(oT[0:64, :], oa[0:64, :],
                                         st["bcs"][:, 0:T])
                    nc.vector.tensor_mul(oT[64:128, :], ob[0:64, :],
                                         st["bcs"][:, T:2 * T])
                    oT_tiles[hp] = oT

                for off, fn in ((1, t1), (2, t2), (3, t25), (4, t3), (5, t4)):
                    tails.setdefault(step + off, []).append(fn)

            load_kt(0)
            load_kt(1)
            load_vts(0)
            for step in range(NIT + 8):
                if step < NIT:
                    stage_scores(step)
                if 1 <= step <= NIT:
                    stage_exp(step - 1)
                if 2 <= step <= NIT + 1:
                    stage_pv(step - 2, step)
                for fn in tails.pop(step, ()):
                    fn()

            out_tiles = [full.tile([128, H], F32, name=out_tag, tag="big")
                         for _ in range(TC)]
            for mt in range(TC):
                for n in range(2):
                    p = ps.tile([128, 512], F32, name="ps", tag="ps")
                    for k in range(HC):
                        nc.tensor.matmul(
                            p[:], oT_tiles[k][:, mt * 128:(mt + 1) * 128],
                            wo_tiles[k][:, n * 512:(n + 1) * 512],
                            start=(k == 0), stop=(k == HC - 1 and bo is None))
                    if bo is not None:
                        nc.tensor.matmul(p[:], ones[:, 0:128],
                                         bo[:, n * 512:(n + 1) * 512],
                                         start=False, stop=True)
                    nc.vector.tensor_add(
                        out_tiles[mt][:, n * 512:(n + 1) * 512], p[:],
                        resid_tiles[mt][:, n * 512:(n + 1) * 512])
            return out_tiles

        # =====================================================================
        # Phase 1: adaLN chain -> hidden_in; self K/V; AllGather(self)
        # =====================================================================
        h_sb, t_sb = [], []
        for mt in range(TC):
            th = full.tile([128, H], F32, name="h", tag="big")
            nc.sync.dma_start(th[:], ext["h"].ap()[mt * 128:(mt + 1) * 128, :])
            h_sb.append(th)
            tt = full.tile([128, H], F32, name="tin", tag="big")
            nc.sync.dma_start(tt[:], ext["t"].ap()[mt * 128:(mt + 1) * 128, :])
            t_sb.append(tt)
        x_sb = load_input("x", "x")

        ag_in_s0 = dram.tile([HALF_LEN], BF16, name="agins0", tag="agins0")
        ag_out_s0 = dram.tile([2 * HALF_LEN], BF16, name="agouts0", tag="agouts0")
        ag_in_s1 = dram.tile([HALF_LEN], BF16, name="agins1", tag="agins1")
        ag_out_s1 = dram.tile([2 * HALF_LEN], BF16, name="agouts1", tag="agouts1")
        ag_in_c = dram.tile([RANK_LEN], BF16, name="aginc", tag="aginc")
        ag_out_c = dram.tile([2 * RANK_LEN], BF16, name="agoutc", tag="agoutc")

        with nc.named_scope("p1_ada"):
            silu = []
            for mt in range(TC):
                nc.vector.tensor_add(t_sb[mt][:], t_sb[mt][:], h_sb[mt][:])
                nc.scalar.activation(t_sb[mt][:], t_sb[mt][:], AF.Silu)
                silu.append(t_sb[mt])
            siluT = transpose_act(silu, "siluT")
            # hn only needs h: compute early to fill PE while ada_w DMA lands
            hn = layernorm(h_sb, "lnh", "hn")
            hnT = transpose_act(hn, "hnT")

            shift_sb = [bmod.tile([128, H], BF16, name="shift", tag="mod") for _ in range(TC)]
            scale1_sb = [bmod.tile([128, H], BF16, name="scale1", tag="mod") for _ in range(TC)]
            for half in range(2):
                ada_tiles = []
                for k in range(HC):
                    t_ = wpool.tile([128, H], BF16, name="ada", tag="w")
                    nc.sync.dma_start(
                        t_[:], ext["ada_w"].ap()[k * 128:(k + 1) * 128,
                                                 half * H:(half + 1) * H])
                    ada_tiles.append(t_)
                for mt in range(TC):
                    for hn_ in range(2):
                        p = ps.tile([128, 512], F32, name="ps", tag="ps")
                        for k in range(HC):
                            nc.tensor.matmul(
                                p[:], siluT[k][:, mt * 128:(mt + 1) * 128],
                                ada_tiles[k][:, hn_ * 512:(hn_ + 1) * 512],
                                start=(k == 0), stop=(k == HC - 1 and not use_bias))
                        if use_bias:
                            nc.tensor.matmul(
                                p[:], ones[:, 0:128],
                                biases["ada_b"][:, (half * 2 + hn_) * 512:
                                                (half * 2 + hn_ + 1) * 512],
                                start=False, stop=True)
                        dst = shift_sb[mt] if half == 0 else scale1_sb[mt]
                        nc.scalar.activation(dst[:, hn_ * 512:(hn_ + 1) * 512], p[:],
                                             AF.Copy, bias=(1.0 if half == 1 else 0.0))

            xln = layernorm(x_sb, None, "xln")
            hin = []
            for mt in range(TC):
                nc.vector.tensor_mul(xln[mt][:], xln[mt][:], scale1_sb[mt][:])
                nc.vector.tensor_add(xln[mt][:], xln[mt][:], shift_sb[mt][:])
                hin.append(xln[mt])
            hinT = transpose_act(hin, "hinT")

        with nc.named_scope("p2_selfkv"):
            wsk = load_weight("Wsk")
            wsv = load_weight("Wsv")
            vaug_s = make_vaug_tiles()
            ksT = [None] * HC
            ag_s = (ag_in_s0, ag_in_s1)
            ag_so = (ag_out_s0, ag_out_s1)
            for halfk in range(2):
                part = proj_T(wsk, hinT, biases.get("bsk"), "ksT",
                              mo_list=range(4 * halfk, 4 * halfk + 4))
                for j, mo in enumerate(range(4 * halfk, 4 * halfk + 4)):
                    ksT[mo] = part[j]
                proj_vaug(wsv, hinT, biases.get("bsv"), vaug_s, n_list=(halfk,))
                agi = ag_s[halfk]
                for j, mo in enumerate(range(4 * halfk, 4 * halfk + 4)):
                    nc.sync.dma_start(
                        agi[j * (128 * T):(j + 1) * (128 * T)]
                        .rearrange("(p f) -> p f", p=128), ksT[mo][:])
                for mt in range(TC):
                    nc.sync.dma_start(
                        agi[KT_H + mt * (128 * (VA_W // 2)):
                            KT_H + (mt + 1) * (128 * (VA_W // 2))]
                        .rearrange("(p f) -> p f", p=128),
                        vaug_s[mt][:, halfk * (VA_W // 2):(halfk + 1) * (VA_W // 2)])
                nc.gpsimd.collective_compute(
                    "AllGather", OP.bypass,
                    replica_groups=[[0, 1], [2, 3], [4, 5], [6, 7]],
                    ins=[agi.opt()], outs=[ag_so[halfk].opt()])

            wsq = load_weight("Wsq")
            qsT = proj_T(wsq, hinT, biases.get("bsq"), "qsT", evict_mul=em_bc)

        # =====================================================================
        # Phase 2: cross K/V from h_n = lnh(h); AllGather(cross)
        # =====================================================================
        with nc.named_scope("p3_crosskv"):
            wck = load_weight("Wck")
            kcT = proj_T(wck, hnT, biases.get("bck"), "kcT")
            wcv = load_weight("Wcv")
            vaug_c = make_vaug_tiles()
            proj_vaug(wcv, hnT, biases.get("bcv"), vaug_c)
            emit_kv(kcT, vaug_c, ag_in_c)
            nc.gpsimd.collective_compute(
                "AllGather", OP.bypass,
                replica_groups=[[0, 1], [2, 3], [4, 5], [6, 7]],
                ins=[ag_in_c.opt()], outs=[ag_out_c.opt()])

        def kt_src_s(hp, sl):
            half, hpl = hp // 4, hp % 4
            return ag_so[half][sl * HALF_LEN + hpl * (128 * T):
                               sl * HALF_LEN + (hpl + 1) * (128 * T)] \
                .rearrange("(p f) -> p f", p=128)

        def vt_src_s(hpp, sl, ro):
            half, hl = hpp // 2, hpp % 2
            v = ag_so[half][sl * HALF_LEN + KT_H:
                            sl * HALF_LEN + KT_H + VA_H] \
                .rearrange("(tt f) -> tt f", tt=T)
            return v[ro:ro + 128, hl * 260:(hl + 1) * 260]

        def kt_src_c(hp, sl):
            return ag_out_c[sl * RANK_LEN + hp * (128 * T):
                            sl * RANK_LEN + (hp + 1) * (128 * T)] \
                .rearrange("(p f) -> p f", p=128)

        def vt_src_c(hpp, sl, ro):
            v = ag_out_c[sl * RANK_LEN + KT_LEN:
                         sl * RANK_LEN + KT_LEN + VA_LEN] \
                .rearrange("(tt f) -> tt f", tt=T)
            return v[ro:ro + 128, hpp * 260:(hpp + 1) * 260]

        # =====================================================================
        # Phase 3: self attention -> hidden_in(+x); cross q
        # =====================================================================
        with nc.named_scope("p4_selfattn"):
            wso = load_weight("Wso")
            hidden_in = emit_attention(qsT, kt_src_s, vt_src_s, wso,
                                       biases.get("bso"), x_sb, "res1")

        with nc.named_scope("p5_ln2q"):
            ln2o = layernorm(hidden_in, "ln2", "ln2o")
            ln2T = transpose_act(ln2o, "ln2T")
            wcq = load_weight("Wcq")
            qcT = proj_T(wcq, ln2T, biases.get("bcq"), "qcT", evict_mul=mk_bc)

        # =====================================================================
        # Phase 4: cross attention -> hidden_mid
        # =====================================================================
        with nc.named_scope("p6_crossattn"):
            wco = load_weight("Wco")
            hidden_mid = emit_attention(qcT, kt_src_c, vt_src_c, wco,
                                        biases.get("bco"), hidden_in, "hmid")

        # =====================================================================
        # Phase 5: FFN + final LN + output
        # =====================================================================
        with nc.named_scope("p7_ffn"):
            ln3o = layernorm(hidden_mid, "ln3", "ln3o")
            hoT = transpose_act(ln3o, "hoT")
            w1 = load_weight("ffn_w1")
            midT = []
            for mo in range(HC):
                p = ps.tile([128, 512], F32, name="ps", tag="ps")
                for k in range(HC):
                    nc.tensor.matmul(p[:], w1[k][:, mo * 128:(mo + 1) * 128],
                                     hoT[k][:], start=(k == 0),
                                     stop=(k == HC - 1 and not use_bias))
                if use_bias:
                    nc.tensor.matmul(
                        p[:], biases["ffn_b1"][:, mo * 128:(mo + 1) * 128],
                        ones[:], start=False, stop=True)
                o = halfT.tile([128, T], BF16, name="midT", tag="ht")
                if mo % 2 == 0:
                    nc.scalar.activation(o[:], p[:], AF.Relu)
                else:
                    nc.vector.tensor_scalar_max(o[:], p[:], 0.0)
                midT.append(o)
            w2 = load_weight("ffn_w2")
            ffres = [full.tile([128, H], F32, name="ffres", tag="big") for _ in range(TC)]
            for mt, n, p in proj_nat_psums(w2, midT, biases.get("ffn_b2")):
                nc.vector.tensor_add(ffres[mt][:, n * 512:(n + 1) * 512], p[:],
                                     ln3o[mt][:, n * 512:(n + 1) * 512])

            lnfo = layernorm(ffres, "lnf", "lnfo")
            for mt in range(TC):
                nc.vector.tensor_add(lnfo[mt][:], lnfo[mt][:], hidden_mid[mt][:])
                nc.sync.dma_start(out_ext.ap()[mt * 128:(mt + 1) * 128, :],
                                  lnfo[mt][:])


def _bf16(a):
    import ml_dtypes
    return np.asarray(a, np.float32).astype(ml_dtypes.bfloat16)


def kernel(**inputs):
    global LAST_RESULT
    use_bias = any(np.any(np.asarray(inputs[nm])) for nm in B_NAMES)
    use_affine = any(
        (not np.array_equal(np.asarray(inputs[nm + "_g"]),
                            np.ones_like(np.asarray(inputs[nm + "_g"])))) or
        np.any(np.asarray(inputs[nm + "_b"]))
        for nm in LN_NAMES)
    flags = (use_bias, use_affine)
    if flags not in _BUILD_CACHE:
        _BUILD_CACHE[flags] = _build(flags)
    nc = _BUILD_CACHE[flags]

    x = np.asarray(inputs["x"], np.float32)
    h = np.asarray(inputs["h"], np.float32)
    t = np.asarray(inputs["t"], np.float32)
    em = np.asarray(inputs["extent_mask"], np.int32)
    mk = np.asarray(inputs["mask"], np.int32)

    common = {}
    for nm in W_NAMES + ["ada_w"]:
        common[nm] = np.ascontiguousarray(_bf16(inputs[nm]))
    for nm in B_NAMES:
        common[nm] = np.ascontiguousarray(_bf16(inputs[nm]).reshape(1, -1))
    for nm in LN_NAMES:
        common[nm + "_g"] = np.asarray(inputs[nm + "_g"], np.float32).reshape(1, -1)
        common[nm + "_b"] = np.asarray(inputs[nm + "_b"], np.float32).reshape(1, -1)
    common["c_ones"] = _bf16(np.ones((1, T)))
    common["c_onesp"] = _bf16(np.ones((128, NH)))
    common["c_ident"] = np.eye(128, dtype=np.float32)

    in_maps = []
    for c in range(NC):
        b, half = c // 2, c % 2
        s0 = half * T
        m = dict(common)
        m["x"] = np.ascontiguousarray(x[b, s0:s0 + T])
        m["h"] = np.ascontiguousarray(h[b, s0:s0 + T])
        m["t"] = np.ascontiguousarray(t[b, s0:s0 + T])
        m["em"] = np.ascontiguousarray(em[b, s0:s0 + T].reshape(1, T))
        m["mk"] = np.ascontiguousarray(mk[b, s0:s0 + T].reshape(1, T))
        in_maps.append(m)

    trace = bool(os.environ.get("BASS_TRACE_KERNEL"))
    if trace:
        _install_ntff_hook()
    try:
        res = bass_utils.run_bass_kernel_spmd(
            nc, in_maps, core_ids=list(range(NC)), trace=trace)
    except Exception:
        import time
        time.sleep(20)
        res = bass_utils.run_bass_kernel_spmd(
            nc, in_maps, core_ids=list(range(NC)), trace=trace)
    LAST_RESULT = res

    out = np.empty((B, S, H), np.float32)
    for c in range(NC):
        b, half = c // 2, c % 2
        out[b, half * T:(half + 1) * T] = res.results[c]["out"]
    return out


def _install_ntff_hook():
    import sys, types
    if 'antenv.axon_hooks' in sys.modules:
        return
    mod = types.ModuleType("antenv.axon_hooks")
    mod._hook = None
    def set_axon_ntff_profile_hook(h): mod._hook = h
    def get_axon_ntff_profile_hook(): return mod._hook
    mod.set_axon_ntff_profile_hook = set_axon_ntff_profile_hook
    mod.get_axon_ntff_profile_hook = get_axon_ntff_profile_hook
    sys.modules['antenv.axon_hooks'] = mod
    import antenv
    antenv.axon_hooks = mod
    try:
        from trn_agent_boot.trn_boot import _ntff_profile_via_ctypes
        mod.set_axon_ntff_profile_hook(
            _ntff_profile_via_ctypes('/opt/axon/libaxon_pjrt.so'))
    except Exception:
        pass
